# revision 51
# baseline (speedup 1.0000x reference)
"""Multi-head self-attention (dense transformer block) on 8 TRN2 NeuronCores.

Data-parallel over batch: 8 batch items -> 8 cores, one image each, zero
collectives.  fp8(e4m3) DoubleRow matmuls for all contraction-bound phases
(QKV projections contract C=512, PV contracts S=1024, output projection
contracts nh*dv=512 padded to 1024).  Scores stay bf16.

v2 restructure (vs the head-serial v1):

* Heads are processed in PAIRS (2m, 2m+1).  K for a pair is packed into one
  kt tile ([0:64] = even head, [64:128] = odd head) so the per-head score
  matmuls contract only 64 partitions and run ROW-TILED (tile_position
  auto-derived from base_partition 0/64): the two heads' score matmuls
  execute concurrently on the PE, and no zero-padding DMAs are needed.
* Per pair-step ki, BOTH heads' score tiles are exp'd concurrently: the
  even head on ScalarE (table exp) and the odd head on the DVE (Schraudolph
  uint8-bits exp), with a few steps flipped to ScalarE where the DVE is
  busy with normalize work.  This roughly halves the exp-chain wall time,
  which paces the whole kernel.
* All PSUM traffic except PV flows through ONE 3-buffer [128,1024] ring
  (6 banks): score tiles, merged QKV projection groups ([128,1024] psum,
  one copy-out op each), and the output-projection tiles at the tail.
  PV keeps its own 2-bank accumulator.
* The PV ones-block trick: v8 template columns 0:64 are all ones, so the
  PV matmul lands the softmax denominator REPLICATED on partitions 0:64 --
  normalize is just reciprocal + multiply, no cross-partition broadcast.
* pv for the even head accumulates at ki 5/6/7 and normalizes at pair end;
  the odd head's pv is deferred into the next pair's early steps (bank WAR
  on the single pv accumulator).  The last pair pipelines pv/normalize/
  output-projection per qc-half to shorten the serial tail.

Scale plan (all power-of-2 so they cancel exactly):
  x8 = x (e4m3), w{q,k,v,o}8 = 16*W (e4m3)
  qt = (Wq8^T x8) * 2^-4   kt = (Wk8^T x8) * 2^-4   (exact-scale bf16)
  v8 = x8^T Wv8 raw (= 16*v) in the templated fp8 tile
       [P kpos, 2 (kpos-chunk pair), NH, 128]: cols 0:64 ones, 64:128 16*v
  est8 = exp(s/8)/4 in e4m3 (ScalarE) or Schraudolph u8 bits (DVE)
  at8 = pv[64:128] / denom = 16*attn (e4m3), at partitions 64:128
  out = (I @ (256*x_bf16) + Wo8^T at8) * 2^-8  -- the residual rides the
  output-projection PSUM accumulation as one extra identity matmul per
  chunk, so the close is a single ScalarE/DVE scale + HWDGE store.

Measured on TRN2 (clean runs): ~98.5-100us vs the 114-120us v1 baseline;
rel err vs the fp32 reference ~5.5e-3 (gate 2e-2).  Chip-level thermal
throttling adds up to ~20% on some runs -- compare structure via per-op
durations (healthy: [128,1024] ScalarE ACT ~1.11us, MM N=512 ~215ns gap).
"""

import math

import numpy as np

B = 8
C = 512
S = 1024
NH = 8
D = 64
P = 128
KO = C // P  # 4 partition tiles over the channel/contract dim
SO = S // P  # 8 partition tiles over positions
NQ = S // 512  # 2 free-dim chunks of 512 per matmul (PSUM bank limit)
NPAIR = NH // 2

_GRAPH_CACHE = {}

# (pair, ki) steps whose ODD-head exp runs on ScalarE instead of the DVE
# (the DVE is busy with the previous pair's normalize work there).
FLIP_B = {(p, ki) for p in range(1, NPAIR) for ki in (0, 3)}


def _build_graph(with_bias: bool):
    import concourse.bass as bass
    import concourse.tile as tile
    from concourse import bacc, mybir
    from contextlib import ExitStack

    F32 = mybir.dt.float32
    BF16 = mybir.dt.bfloat16
    F8 = mybir.dt.float8e4
    U8 = mybir.dt.uint8
    Exp = mybir.ActivationFunctionType.Exp
    ADD = mybir.AluOpType.add
    MUL = mybir.AluOpType.mult
    DR = mybir.MatmulPerfMode.DoubleRow
    SCH_A = float(8.0 * math.log2(math.e) / 8.0)  # 1.442695
    SCH_B = 40.0
    EXP_BIAS = -2.0 * math.log(2.0)

    nc = bacc.Bacc("TRN2", target_bir_lowering=False, debug=False, num_devices=B)

    xb16 = nc.declare_dram_parameter("xb16", [P, KO, S], BF16, isOutput=False)
    ident = nc.declare_dram_parameter("ident", [P, P], BF16, isOutput=False)
    x8 = nc.declare_dram_parameter("x8", [P, KO, S], F8, isOutput=False)
    wq8 = nc.declare_dram_parameter("wq8", [P, KO, NH * D], F8, isOutput=False)
    wk8 = nc.declare_dram_parameter("wk8", [P, KO, NH * D], F8, isOutput=False)
    wv8 = nc.declare_dram_parameter("wv8", [P, KO, NH * D], F8, isOutput=False)
    wo8 = nc.declare_dram_parameter("wo8", [P, 2 * KO, C], F8, isOutput=False)
    if with_bias:
        bq = nc.declare_dram_parameter("bq", [NH * D], F32, isOutput=False)
        bk = nc.declare_dram_parameter("bk", [NH * D], F32, isOutput=False)
        bv16 = nc.declare_dram_parameter("bv16", [NH * D], F32, isOutput=False)
        bo = nc.declare_dram_parameter("bo", [C], F32, isOutput=False)
    out = nc.declare_dram_parameter("out", [C, S], F32, isOutput=True)

    with ExitStack() as ctx:
        tc = ctx.enter_context(tile.TileContext(nc))
        singles = ctx.enter_context(tc.tile_pool(name="singles", bufs=1))
        est_po = ctx.enter_context(tc.tile_pool(name="est_po", bufs=14))
        out_po = ctx.enter_context(tc.tile_pool(name="out_po", bufs=4))
        rr_po = ctx.enter_context(tc.tile_pool(name="rr_po", bufs=2))

        x8_sb = singles.tile([P, KO, S], F8, tag="x8", name="x8")
        wq_sb = singles.tile([P, KO, NH * D], F8, tag="wq", name="wq")
        wk_sb = singles.tile([P, KO, NH * D], F8, tag="wk", name="wk")
        wv_sb = singles.tile([P, KO, NH * D], F8, tag="wv", name="wv")
        wo_sb = singles.tile([P, 2 * KO, C], F8, tag="wo", name="wo")
        qt_sb = [singles.tile([P, S], BF16, tag=f"qt{m}", name=f"qt{m}") for m in range(KO)]
        # kt pair-packed: rows 0:64 = head 2m, rows 64:128 = head 2m+1
        kt_sb = [singles.tile([P, S], BF16, tag=f"kt{m}", name=f"kt{m}") for m in range(KO)]
        v8a = singles.tile([P, SO, NH, 2 * D], F8, tag="v8a", name="v8a")
        v8_sb = [v8a[:, 2 * p : 2 * p + 2] for p in range(SO // 2)]
        at8a = singles.tile([P, KO, 2, S], F8, tag="at8a", name="at8a")
        at8_sb = [at8a[:, t] for t in range(KO)]

        # ---- loads.  x8 alone on the scalar queue gates the first matmul;
        # wq/wk lead the sync queue.  Everything else follows.
        # x8 halves on two queues in parallel -> first matmul ~1.5us earlier
        nc.scalar.dma_start(out=x8_sb[:, 0:2], in_=x8[:, 0:2])
        nc.sync.dma_start(out=x8_sb[:, 2:KO], in_=x8[:, 2:KO])
        nc.sync.dma_start(out=wq_sb[:], in_=wq8[:])
        nc.sync.dma_start(out=wk_sb[:], in_=wk8[:])
        nc.sync.dma_start(out=wv_sb[:], in_=wv8[:])
        nc.sync.dma_start(out=wo_sb[:], in_=wo8[:])
        # residual (256*x in bf16) + identity: folded into the output
        # projection's PSUM accumulation via one extra matmul per chunk
        xb_sb = singles.tile([P, KO, S], BF16, tag="xb16", name="xb16")
        id_sb = singles.tile([P, P], BF16, tag="ident", name="ident")
        nc.sync.dma_start(out=id_sb[:], in_=ident[:])
        nc.sync.dma_start(out=xb_sb[:], in_=xb16[:])
        # V template ones-block + at8 zero rows: two big memsets
        nc.gpsimd.memset(v8a[:, :, :, 0:D], 1.0)
        nc.gpsimd.memset(at8a[0:D, :, :, :], 0.0)

        ebias = singles.tile([P, 1], F32, tag="ebias")
        nc.vector.memset(ebias[:], EXP_BIAS)
        # HAM warm-up: the PE otherwise idles until the x8/wq DMAs land and
        # then runs the whole lead-in at the cold 1.2 GHz clock (the 4096-
        # cycle activity window only trips ~10us later).  A dense burst of
        # dummy matmuls on a zeroed tile trips it before the first real MM.
        # The scratch bank is safe: every later accumulation group in the
        # ring opens with start=True, which clears it.
        wtile = singles.tile([P, 512], BF16, tag="warm")
        nc.vector.memset(wtile[:], 0.0)

        if with_bias:
            bq_sb = singles.tile([P, KO, 1], F32, tag="bq")
            bk_sb = singles.tile([P, KO, 1], F32, tag="bk")
            nc.sync.dma_start(out=bq_sb[:, :, 0], in_=bq.rearrange("(ko p) -> p ko", p=P))
            nc.sync.dma_start(out=bk_sb[:, :, 0], in_=bk.rearrange("(ko p) -> p ko", p=P))
            bv_rep = singles.tile([P, NH * D], F32, tag="bv")
            _bv_ap = bv16.ap()
            nc.sync.dma_start(
                out=bv_rep[:],
                in_=bass.AP(
                    tensor=_bv_ap.tensor, offset=_bv_ap.offset, ap=[[0, P], [1, NH * D]]
                ),
            )
            bo_sb = singles.tile([P, KO, 1], F32, tag="bo")
            nc.sync.dma_start(out=bo_sb[:, :, 0], in_=bo.rearrange("(ko p) -> p ko", p=P))

        # PSUM: one [128,1024] ring (6 banks) + the pv accumulator (2 banks)
        ps_ctx = tc.tile_pool(name="ps_ring", bufs=3, space="PSUM")
        pv_ctx = tc.tile_pool(name="pv_ps", bufs=1, space="PSUM")
        ps_ring = ps_ctx.__enter__()
        pv_ps = pv_ctx.__enter__()

        def ring_tile(name):
            return ps_ring.tile([P, S], F32, tag="ps", name=name)

        # 16 dummies bridge from engine boot (~3.6us) to the x8/wq DMA
        # landing (~8.5us) with no PE-idle window: ~8 run at the cold clock
        # (3.4us, tripping HAM), the rest at 2.4 GHz.
        warm = ring_tile("warmup")
        for i in range(16):
            nc.tensor.matmul(
                warm[:, 0:512], wtile[:, 0:P], wtile[:], start=True, stop=True
            )

        # ---------- projection groups (merged [128,1024] psum).  The MMs
        # issue at their weave step; the copy-out is RETURNED as a closure
        # and issued one step later (after the next exps), so it never
        # head-blocks the exp engines' strict FIFOs while the PE drains.
        def q_proj(mo, eng):
            ps = ring_tile(f"pjq{mo}")
            for qc in range(NQ):
                for j in range(2):
                    nc.tensor.matmul(
                        ps[:, qc * 512 : (qc + 1) * 512],
                        wq_sb[:, 2 * j : 2 * j + 2, mo * P : (mo + 1) * P],
                        x8_sb[:, 2 * j : 2 * j + 2, qc * 512 : (qc + 1) * 512],
                        start=(j == 0),
                        stop=(j == 1),
                        perf_mode=DR,
                    )

            def copy():
                if with_bias:
                    nc.vector.tensor_scalar(
                        out=qt_sb[mo][:], in0=ps[:], scalar1=1.0 / 16.0,
                        scalar2=bq_sb[:, mo], op0=MUL, op1=ADD,
                    )
                elif eng == "s":
                    nc.scalar.mul(qt_sb[mo][:], ps[:], 1.0 / 16.0)
                else:
                    nc.vector.tensor_scalar_mul(
                        out=qt_sb[mo][:], in0=ps[:], scalar1=1.0 / 16.0
                    )

            return copy

        def k_proj(mo, eng):
            ps = ring_tile(f"pjk{mo}")
            for qc in range(NQ):
                for j in range(2):
                    nc.tensor.matmul(
                        ps[:, qc * 512 : (qc + 1) * 512],
                        wk_sb[:, 2 * j : 2 * j + 2, mo * P : (mo + 1) * P],
                        x8_sb[:, 2 * j : 2 * j + 2, qc * 512 : (qc + 1) * 512],
                        start=(j == 0),
                        stop=(j == 1),
                        perf_mode=DR,
                    )

            def copy():
                if with_bias:
                    nc.vector.tensor_scalar(
                        out=kt_sb[mo][:], in0=ps[:], scalar1=1.0 / 16.0,
                        scalar2=bk_sb[:, mo], op0=MUL, op1=ADD,
                    )
                elif eng == "s":
                    nc.scalar.mul(kt_sb[mo][:], ps[:], 1.0 / 16.0)
                else:
                    nc.vector.tensor_scalar_mul(
                        out=kt_sb[mo][:], in0=ps[:], scalar1=1.0 / 16.0
                    )

            return copy

        def v_proj(sp, eng):
            # so = 2*sp, 2*sp+1 merged into one [128,1024] group
            ps = ring_tile(f"pjv{sp}")
            for half in range(2):
                so = 2 * sp + half
                for j in range(2):
                    nc.tensor.matmul(
                        ps[:, half * 512 : (half + 1) * 512],
                        x8_sb[:, 2 * j : 2 * j + 2, so * P : (so + 1) * P],
                        wv_sb[:, 2 * j : 2 * j + 2, :],
                        start=(j == 0),
                        stop=(j == 1),
                        perf_mode=DR,
                    )

            def copy():
                dst = v8_sb[sp][:, :, :, D : 2 * D]
                src = ps[:].rearrange("p (c h d) -> p c h d", c=2, h=NH)
                if with_bias:
                    bvr = bv_rep[:].rearrange("p (h d) -> p h d", h=NH)
                    for half in range(2):
                        nc.vector.tensor_tensor(dst[:, half], src[:, half], bvr, ADD)
                elif eng == "s":
                    nc.scalar.copy(dst, src)
                else:
                    nc.vector.tensor_copy(out=dst, in_=src)

            return copy

        # ---------- scores: row-tiled 64-contraction matmuls.  The A and B
        # halves of a step are issued at different points of the previous
        # step (A early, B after the heavy PE work) so the PE FIFO never
        # starves the exp engines waiting behind pv/projection blocks.
        def st_half(m, ki, half, name):
            st = ring_tile(name)
            lo = half * D
            for qc in range(NQ):
                nc.tensor.matmul(
                    st[:, qc * 512 : (qc + 1) * 512],
                    kt_sb[m][lo : lo + D, ki * P : (ki + 1) * P],
                    qt_sb[m][lo : lo + D, qc * 512 : (qc + 1) * 512],
                    start=True,
                    stop=True,
                )
            return st

        def exp_tile(st, eslot, eng):
            if eng == "s":
                nc.scalar.activation(
                    out=eslot, in_=st[:], func=Exp, scale=1.0 / 8.0,
                    bias=ebias[:, 0:1],
                )
            else:
                nc.vector.tensor_scalar(
                    out=eslot.bitcast(U8), in0=st[:], scalar1=SCH_A,
                    scalar2=SCH_B, op0=MUL, op1=ADD,
                )

        def pv_chunks(h, pv_t, est_h, chunks, qcs=(0, 1)):
            for pch in chunks:
                for qc in qcs:
                    nc.tensor.matmul(
                        pv_t[:, qc * 512 : (qc + 1) * 512],
                        v8_sb[pch][:, :, h, :],
                        est_h[pch][:, :, qc * 512 : (qc + 1) * 512],
                        start=(pch == 0),
                        stop=(pch == SO // 2 - 1),
                        perf_mode=DR,
                    )

        def pv_half(h, dst, est_h, qc):
            # one qc-half of pv in its own ring tile (independent WAR)
            for pch in range(SO // 2):
                nc.tensor.matmul(
                    dst[:, 0:512],
                    v8_sb[pch][:, :, h, :],
                    est_h[pch][:, :, qc * 512 : (qc + 1) * 512],
                    start=(pch == 0),
                    stop=(pch == SO // 2 - 1),
                    perf_mode=DR,
                )

        def normalize_half(h, pvh, qc):
            t, j = h // 2, h % 2
            sl = slice(qc * 512, (qc + 1) * 512)
            rrep = rr_po.tile([D, 512], F32, tag="rrepH")
            nc.vector.reciprocal_approx_fast(out=rrep[:], in_=pvh[0:D, 0:512])
            nc.vector.tensor_tensor(
                at8_sb[t][D:P, j, sl], pvh[D:P, 0:512], rrep[:], MUL
            )

        def normalize(h, pv_t, qcs=None):
            # pv rows 0:64 = denominator (replicated), 64:128 = 16*attnT
            t, j = h // 2, h % 2
            if qcs is None:  # full width, one recip + one multiply
                rrep = rr_po.tile([D, S], F32, tag="rrepF")
                nc.vector.reciprocal_approx_fast(out=rrep[:], in_=pv_t[0:D, :])
                nc.vector.tensor_tensor(
                    at8_sb[t][D:P, j, :], pv_t[D:P, :], rrep[:], MUL
                )
                return
            for qc in qcs:
                sl = slice(qc * 512, (qc + 1) * 512)
                rrep = rr_po.tile([D, 512], F32, tag="rrepH")
                nc.vector.reciprocal_approx_fast(out=rrep[:], in_=pv_t[0:D, sl])
                nc.vector.tensor_tensor(
                    at8_sb[t][D:P, j, sl], pv_t[D:P, sl], rrep[:], MUL
                )

        # ---------- output projection chunk ([128,1024] ring tile, mo row)
        out_r = out.rearrange("(mo p) s -> p mo s", p=P)

        _oq = [nc.scalar, nc.sync]

        def po_open(mo, ts):
            # the residual matmul (identity @ 256*x_bf16) opens the
            # accumulation group; the wo matmuls pile attn*256 on top
            ps = ring_tile(f"po{mo}")
            for qc in range(NQ):
                nc.tensor.matmul(
                    ps[:, qc * 512 : (qc + 1) * 512],
                    id_sb[:],
                    xb_sb[:, mo, qc * 512 : (qc + 1) * 512],
                    start=True,
                    stop=False,
                )
            for t in ts:
                for qc in range(NQ):
                    nc.tensor.matmul(
                        ps[:, qc * 512 : (qc + 1) * 512],
                        wo_sb[:, 2 * t : 2 * t + 2, mo * P : (mo + 1) * P],
                        at8_sb[t][:, :, qc * 512 : (qc + 1) * 512],
                        start=False,
                        stop=(t == KO - 1),
                        perf_mode=DR,
                    )
            return ps

        def po_finish(ps, mo, ts, qcs=(0, 1)):
            for t in ts:
                for qc in qcs:
                    nc.tensor.matmul(
                        ps[:, qc * 512 : (qc + 1) * 512],
                        wo_sb[:, 2 * t : 2 * t + 2, mo * P : (mo + 1) * P],
                        at8_sb[t][:, :, qc * 512 : (qc + 1) * 512],
                        start=False,
                        stop=(t == KO - 1),
                        perf_mode=DR,
                    )

        def po_close(ps, mo, qcs=(0, 1)):
            # closes alternate ScalarE/DVE (both idle at the tail)
            Copy = mybir.ActivationFunctionType.Copy
            for qc in qcs:
                sl = slice(qc * 512, (qc + 1) * 512)
                ot = out_po.tile([P, 512], F32, tag="ot")
                if with_bias:
                    nc.scalar.activation(
                        out=ot[:], in_=ps[:, sl], func=Copy, scale=1.0 / 256.0,
                        bias=bo_sb[:, mo],
                    )
                elif (mo + qc) % 2 == 0:
                    nc.scalar.mul(ot[:], ps[:, sl], 1.0 / 256.0)
                else:
                    nc.vector.tensor_scalar_mul(out=ot[:], in0=ps[:, sl], scalar1=1.0 / 256.0)
                _oq[(mo + qc) % 2].dma_start(out=out_r[:, mo, sl], in_=ot[:])

        # ================= lead-in =================
        # q0/k0 copy-outs split into qc halves: the first score matmuls
        # (which read kt cols 0:128 and qt half 0) unblock on the first
        # half instead of the full [128,1024] copy.
        if with_bias:
            q_proj(0, "v")()
            k_proj(0, "s")()
        else:
            psq = ring_tile("pjq0")
            for qc in range(NQ):
                for j in range(2):
                    nc.tensor.matmul(
                        psq[:, qc * 512 : (qc + 1) * 512],
                        wq_sb[:, 2 * j : 2 * j + 2, 0:P],
                        x8_sb[:, 2 * j : 2 * j + 2, qc * 512 : (qc + 1) * 512],
                        start=(j == 0),
                        stop=(j == 1),
                        perf_mode=DR,
                    )
            nc.vector.tensor_scalar_mul(
                out=qt_sb[0][:, 0:512], in0=psq[:, 0:512], scalar1=1.0 / 16.0
            )
            psk = ring_tile("pjk0")
            for qc in range(NQ):
                for j in range(2):
                    nc.tensor.matmul(
                        psk[:, qc * 512 : (qc + 1) * 512],
                        wk_sb[:, 2 * j : 2 * j + 2, 0:P],
                        x8_sb[:, 2 * j : 2 * j + 2, qc * 512 : (qc + 1) * 512],
                        start=(j == 0),
                        stop=(j == 1),
                        perf_mode=DR,
                    )
            nc.vector.tensor_scalar_mul(
                out=qt_sb[0][:, 512:S], in0=psq[:, 512:S], scalar1=1.0 / 16.0
            )
            nc.scalar.mul(kt_sb[0][:, 0:512], psk[:, 0:512], 1.0 / 16.0)
            nc.scalar.mul(kt_sb[0][:, 512:S], psk[:, 512:S], 1.0 / 16.0)

        # weave plan: (pair, ki) -> list of callables issued after that step's
        # exps.  Copies alternate engines to balance the exp load.
        weave = {
            (0, 0): [lambda: v_proj(0, "v")],
            (0, 1): [lambda: q_proj(1, "s")],
            (0, 2): [lambda: v_proj(1, "s")],
            (0, 3): [lambda: k_proj(1, "s")],
            (0, 4): [lambda: v_proj(2, "v")],
            (0, 5): [lambda: v_proj(3, "s")],
            (1, 1): [lambda: q_proj(2, "s")],
            (1, 3): [lambda: k_proj(2, "s")],
            (2, 1): [lambda: q_proj(3, "s")],
            (2, 3): [lambda: k_proj(3, "s")],
        }

        # ================= paired attention loop =================
        # software-pipelined: step gs's score tiles are issued during step
        # gs-1 (the A half early -- its ring slot is long free -- and the B
        # half after the pv/projection PE work, by which time the ScalarE
        # exp whose bank it reuses has retired).
        GT = NPAIR * SO
        est = {}  # est[h][pch] tiles
        pv_tiles = {}
        po_ps = {}
        sts = {0: (st_half(0, 0, 0, "stA0_0"), st_half(0, 0, 1, "stB0_0"))}
        pending = []
        for gs in range(GT):
            p, ki = divmod(gs, SO)
            hA, hB = 2 * p, 2 * p + 1
            if ki == 0:
                est[hA] = {}
                est[hB] = {}
            stA, stB = sts.pop(gs)
            if ki % 2 == 0:
                est[hA][ki // 2] = est_po.tile([P, 2, S], F8, tag="est", name=f"estA{p}_{ki}")
                est[hB][ki // 2] = est_po.tile([P, 2, S], F8, tag="est", name=f"estB{p}_{ki}")
            eslotA = est[hA][ki // 2][:, ki % 2, :]
            eslotB = est[hB][ki // 2][:, ki % 2, :]
            exp_tile(stA, eslotA, "s")
            exp_tile(stB, eslotB, "s" if (p, ki) in FLIP_B else "v")

            # last step's deferred projection copy-outs (their MMs have
            # long drained from the PE by now)
            for fn in pending:
                fn()
            pending = []

            nxt = divmod(gs + 1, SO) if gs + 1 < GT else None

            # deferred pv of the PREVIOUS pair's odd head
            if p > 0:
                if ki == 2:
                    pv_tiles[hB - 2] = pv_ps.tile([P, S], F32, tag="pv", name=f"pv{hB - 2}")
                    pv_chunks(hB - 2, pv_tiles[hB - 2], est[hB - 2], (0, 1))
                elif ki == 3:
                    pv_chunks(hB - 2, pv_tiles[hB - 2], est[hB - 2], (2, 3))
                    normalize(hB - 2, pv_tiles[hB - 2])

            # this pair's even head pv at ki 5/6/7
            if ki == 5:
                pv_tiles[hA] = pv_ps.tile([P, S], F32, tag="pv", name=f"pv{hA}")
                pv_chunks(hA, pv_tiles[hA], est[hA], (0, 1))
            elif ki == 6:
                pv_chunks(hA, pv_tiles[hA], est[hA], (2,))
            elif ki == 7:
                pv_chunks(hA, pv_tiles[hA], est[hA], (3,))
                normalize(hA, pv_tiles[hA])
                if p > 0:
                    del est[hB - 2]
                del est[hA]

            for fn in weave.get((p, ki), ()):
                r = fn()
                if r is not None:
                    r()  # copy-out immediately (deferring measured slower)

            # next step's pair last, ADJACENT so the row-tiled halves run
            # concurrently; by now the exps their ring slots WAR on are done
            if nxt:
                nA = st_half(nxt[0], nxt[1], 0, f"stA{nxt[0]}_{nxt[1]}")
                nB = st_half(nxt[0], nxt[1], 1, f"stB{nxt[0]}_{nxt[1]}")
                sts[gs + 1] = (nA, nB)

        # ================= tail: last head's pv halves in TWO ring tiles so
        # the qc1 accumulation never WARs on the qc0 normalize; po chunks
        # flow through the freed ring slots.
        hL = NH - 1  # head 7
        pq0 = ring_tile("pvLq0")
        pv_half(hL, pq0, est[hL], 0)
        pq1 = ring_tile("pvLq1")
        pv_half(hL, pq1, est[hL], 1)
        po_ps[0] = po_open(0, (0, 1, 2))
        normalize_half(hL, pq0, 0)
        po_ps[1] = po_open(1, (0, 1, 2))
        normalize_half(hL, pq1, 1)
        po_finish(po_ps[0], 0, (3,), qcs=(0,))
        po_close(po_ps[0], 0, qcs=(0,))
        po_finish(po_ps[1], 1, (3,), qcs=(0,))
        po_close(po_ps[1], 1, qcs=(0,))
        po_finish(po_ps[0], 0, (3,), qcs=(1,))
        po_close(po_ps[0], 0, qcs=(1,))
        po_ps[2] = po_open(2, (0, 1, 2, 3))
        po_finish(po_ps[1], 1, (3,), qcs=(1,))
        po_close(po_ps[1], 1, qcs=(1,))
        po_close(po_ps[2], 2)
        po_ps[3] = po_open(3, (0, 1, 2, 3))
        po_close(po_ps[3], 3)

        pv_ctx.__exit__(None, None, None)
        ps_ctx.__exit__(None, None, None)

    nc.compile()
    return nc


def _get_graph(with_bias: bool):
    key = bool(with_bias)
    if key not in _GRAPH_CACHE:
        _GRAPH_CACHE[key] = _build_graph(key)
    return _GRAPH_CACHE[key]


def _make_in_maps(inputs, with_bias: bool):
    import ml_dtypes

    e4 = np.dtype(ml_dtypes.float8_e4m3fn)
    f32 = np.float32

    def to8(a):
        return np.ascontiguousarray(np.clip(a, -240.0, 240.0).astype(e4))

    x = np.ascontiguousarray(np.asarray(inputs["x"], dtype=f32))
    assert x.shape == (B, C, 32, 32), x.shape
    xf = x.reshape(B, C, S)
    # x8[p, ko, s] = x[ko*128+p, s]
    x8 = xf.reshape(B, KO, P, S).transpose(0, 2, 1, 3)

    def wre(w):  # [C, N] -> [P, KO, N] with c = ko*128+p, scaled by 16
        a = np.asarray(w, dtype=f32) * 16.0
        return to8(a.reshape(KO, P, -1).transpose(1, 0, 2))

    ws = {
        "wq8": wre(inputs["Wq"]),
        "wk8": wre(inputs["Wk"]),
        "wv8": wre(inputs["Wv"]),
    }
    # wo8[p, s, c] = 16*Wo[s*64 + (p-64), c] for p >= 64 else 0
    wo = np.asarray(inputs["Wo"], dtype=f32) * 16.0  # [NH*D, C]
    wo8 = np.zeros((P, 2 * KO, C), dtype=f32)
    wo8[D:P, :, :] = wo.reshape(2 * KO, D, C).transpose(1, 0, 2)
    ws["wo8"] = to8(wo8)
    bf16 = np.dtype(ml_dtypes.bfloat16)
    ws["ident"] = np.ascontiguousarray(np.eye(P, dtype=bf16))

    maps = []
    for b in range(B):
        m = {
            "xb16": np.ascontiguousarray((x8[b] * 256.0).astype(bf16)),
            "x8": to8(x8[b]),
        }
        m.update(ws)
        if with_bias:
            m["bq"] = np.ascontiguousarray(np.asarray(inputs["bq"], dtype=f32))
            m["bk"] = np.ascontiguousarray(np.asarray(inputs["bk"], dtype=f32))
            m["bv16"] = np.ascontiguousarray(np.asarray(inputs["bv"], dtype=f32) * 16.0)
            m["bo"] = np.ascontiguousarray(np.asarray(inputs["bo"], dtype=f32))
        maps.append(m)
    return maps


def _run(inputs, **spmd_kwargs):
    from concourse.bass_utils import run_bass_kernel_spmd

    nh = int(np.asarray(inputs.get("num_heads", NH)))
    assert nh == NH, f"kernel hardcodes num_heads={NH}, got {nh}"
    with_bias = any(
        np.any(np.asarray(inputs[k])) for k in ("bq", "bk", "bv", "bo") if k in inputs
    )
    nc = _get_graph(with_bias)
    in_maps = _make_in_maps(inputs, with_bias)
    res = run_bass_kernel_spmd(nc, in_maps, core_ids=list(range(B)), **spmd_kwargs)
    outs = np.stack([res.results[b]["out"] for b in range(B)])  # [B, C, S]
    return outs.reshape(B, C, 32, 32).astype(np.float32), res


def kernel(**inputs):
    out, _ = _run(inputs)
    return out
